# revision 36
# baseline (speedup 1.0000x reference)
"""Trainium2 Bass kernel for nn_AdaptiveAttention (dense_cnn).

Math (per image, C=256, H=W=128):
    avg = mean(x, spatial); mx = max(x, spatial)             [C]
    ca  = sigmoid(fc(avg) + fc(mx))                          [C]   (tiny MLP+BN)
    g   = sigmoid(gate_w . x + gate_b)                       [H,W]
    s   = sigmoid(conv7x7([mean_c(x), max_c(x)]) + sa_b)     [H,W]
    out = x*(1 + g*(alpha*ca + 0.1*alpha + beta*s)) - g*(0.1*alpha*avg)
        = x * t - g * D     with t = A_c*g + q1,  q1 = 1 + beta*g*s,
          A_c = alpha*ca_c + 0.1*alpha,  D_c = 0.1*alpha*avg_c

Distribution: pure data-parallel, 2 images per NeuronCore across 8 cores.
Core compute in bf16 (DVE 2x modes), stats accumulated in f32.
"""
import numpy as np
from contextlib import ExitStack

import concourse.bass as bass
import concourse.bacc as bacc
import concourse.mybir as mybir
import concourse.tile as tile
from concourse.bass_utils import run_bass_kernel_spmd

# ---- problem constants (hardcoded per spec) ----
B, C, H, W = 16, 256, 128, 128
NCORES = 8
BLOC = B // NCORES        # 2 images per core
HW = H * W                # 16384 pixels
P = 128                   # partitions
NCT = C // P              # 2 channel tiles
HID = 16
NCH = 2048                # pixels per chunk
NCHUNK = HW // NCH        # 8
EPS = 1e-5

f32 = mybir.dt.float32
bf16 = mybir.dt.bfloat16
AL = mybir.AluOpType
AF = mybir.ActivationFunctionType
AX = mybir.AxisListType


def build_nc():
    nc = bacc.Bacc()

    # ---- DRAM parameters ----
    x_ext = nc.declare_dram_parameter("x", [BLOC, C, HW], f32, isOutput=False)
    out_ext = nc.declare_dram_parameter("out", [BLOC, C, HW], f32, isOutput=True)
    # host-prepped parameters (see make_in_maps)
    og_ext = nc.declare_dram_parameter("og", [C, 2], f32, isOutput=False)
    w1ta_ext = nc.declare_dram_parameter("w1t_avg", [C, HID], f32, isOutput=False)
    w1t_ext = nc.declare_dram_parameter("w1t", [C, HID], f32, isOutput=False)
    w2t_ext = nc.declare_dram_parameter("w2t", [HID, C], f32, isOutput=False)
    mlpc_ext = nc.declare_dram_parameter("mlp_cols", [HID, 3], f32, isOutput=False)
    bnc_ext = nc.declare_dram_parameter("bn_cols", [C, 2], f32, isOutput=False)
    saw_ext = nc.declare_dram_parameter("sa_wp", [1, 98], f32, isOutput=False)
    # sc_par: [alpha, 0.1*alpha, beta, gate_b, sa_b, -0.1*alpha/HW]
    scp_ext = nc.declare_dram_parameter("sc_par", [1, 6], f32, isOutput=False)

    # DRAM scratch for per-pixel rows (g, q1) used for partition-broadcast
    rows_dram = nc.dram_tensor("rows_scratch", [BLOC, 2, HW], bf16)
    # DRAM scratch for the channel-max map rearrange
    rrt_dram = nc.dram_tensor("rrt_scratch", [BLOC, 32, 512], bf16)
    # DRAM scratch for channel-sum / gate-logit row reshapes
    cgrow_dram = nc.dram_tensor("cgrow_scratch", [BLOC, NCHUNK, 2, NCH], bf16)

    x_r = x_ext[:].rearrange("b (t p) n -> b t p n", p=P)
    out_r = out_ext[:].rearrange("b (t p) n -> b t p n", p=P)

    with tile.TileContext(nc) as tc, ExitStack() as ctx:
        const = ctx.enter_context(tc.tile_pool(name="const", bufs=1))
        stats = ctx.enter_context(tc.tile_pool(name="stats", bufs=2))
        maps = ctx.enter_context(tc.tile_pool(name="maps", bufs=2))
        xf_pool = ctx.enter_context(tc.tile_pool(name="xf", bufs=2))
        xb_pool = ctx.enter_context(tc.tile_pool(name="xb", bufs=2 * NCHUNK + 3))
        m1_pool = ctx.enter_context(tc.tile_pool(name="m1", bufs=1))
        rows_pool = ctx.enter_context(tc.tile_pool(name="rows", bufs=1))
        bc_pool = ctx.enter_context(tc.tile_pool(name="bc", bufs=2))
        work = ctx.enter_context(tc.tile_pool(name="work", bufs=2))
        of_pool = ctx.enter_context(tc.tile_pool(name="of", bufs=2))
        ps_cg = ctx.enter_context(tc.tile_pool(name="pscg", bufs=1, space="PSUM"))
        ps_mlp = ctx.enter_context(tc.tile_pool(name="psmlp", bufs=2, space="PSUM"))

        # ================= init: constants =================
        og = []
        for ct in range(NCT):
            of32 = const.tile([P, 2], f32, tag=f"ogf{ct}", name=f"ogf{ct}")
            nc.sync.dma_start(of32[:], og_ext[ct * P:(ct + 1) * P, :])
            o = const.tile([P, 2], bf16, tag=f"og{ct}", name=f"og{ct}")
            nc.vector.tensor_copy(o[:], of32[:])
            og.append(o)

        w1T, w1Ts, w2T = [], [], []
        for ct in range(NCT):
            cs = slice(ct * P, (ct + 1) * P)
            t = const.tile([P, HID], f32, tag=f"w1T{ct}", name=f"w1T{ct}")
            nc.sync.dma_start(t[:], w1t_ext[cs, :])
            ts_ = const.tile([P, HID], f32, tag=f"w1Ts{ct}", name=f"w1Ts{ct}")
            nc.sync.dma_start(ts_[:], w1ta_ext[cs, :])
            w2 = const.tile([HID, P], f32, tag=f"w2T{ct}", name=f"w2T{ct}")
            nc.sync.dma_start(w2[:], w2t_ext[:, cs])
            w1T.append(t)
            w1Ts.append(ts_)
            w2T.append(w2)

        mlpc = const.tile([HID, 3], f32, tag="mlpc", name="mlpc")
        nc.sync.dma_start(mlpc[:], mlpc_ext[:])
        p1mp2 = mlpc[:, 0:1]
        p2c = mlpc[:, 1:2]
        acbc = mlpc[:, 2:3]

        bnscale, bnbias = [], []
        for ct in range(NCT):
            cs = slice(ct * P, (ct + 1) * P)
            bc2 = const.tile([P, 2], f32, tag=f"bnc{ct}", name=f"bnc{ct}")
            nc.sync.dma_start(bc2[:], bnc_ext[cs, :])
            bnscale.append(bc2[:, 0:1])
            bnbias.append(bc2[:, 1:2])

        # broadcast columns [128, 1] from sc_par and the conv weights
        scp = const.tile([P, 6], f32, tag="scp", name="scp")
        nc.sync.dma_start(scp[:], scp_ext[:].to_broadcast([P, 6]))
        alpha_col = scp[:, 0:1]
        alpha01 = scp[:, 1:2]
        beta_col = scp[:, 2:3]
        gateb_col = scp[:, 3:4]
        sab_col = scp[:, 4:5]
        dnegs_col = scp[:, 5:6]
        sa_f32 = const.tile([P, 98], f32, tag="sa_f32", name="sa_f32")
        nc.sync.dma_start(sa_f32[:], saw_ext[:].to_broadcast([P, 98]))
        sa_cols = const.tile([P, 98], bf16, tag="sa_cols", name="sa_cols")
        nc.vector.tensor_copy(sa_cols[:], sa_f32[:])

        # ================= per-image pipeline =================
        for b in range(BLOC):
            ssum_part = [stats.tile([P, NCHUNK], f32, tag=f"ssum{ct}", name=f"ssum{ct}")
                         for ct in range(NCT)]
            rmax = stats.tile([P, NCHUNK * 64], bf16, tag="rmax", name="rmax")
            mf = [stats.tile([P, NCH // 2], bf16, tag=f"mf{ct}", name=f"mf{ct}")
                  for ct in range(NCT)]
            for ct in range(NCT):
                nc.vector.memset(mf[ct][:], -3.0e38)
            cs_hw = maps.tile([P, W], bf16, tag="cs_hw", name="cs_hw")
            glog_hw = maps.tile([P, W], bf16, tag="glog_hw", name="glog_hw")

            xb = [[None] * NCHUNK for _ in range(NCT)]

            # ---- stats pass over chunks ----
            for k in range(NCHUNK):
                ks = slice(k * NCH, (k + 1) * NCH)
                for ct in range(NCT):
                    xf = xf_pool.tile([P, NCH], f32, tag="xf", name="xf")
                    nc.sync.dma_start(xf[:], x_r[b, ct, :, ks])
                    xt = xb_pool.tile([P, NCH], bf16, tag="xb", name="xb")
                    # fp32->bf16 convert; accumulate fp32 spatial sum for free
                    nc.scalar.activation(
                        out=xt[:], in_=xf[:], func=AF.Copy,
                        accum_out=ssum_part[ct][:, k:k + 1])
                    # spatial max: fold then accumulate (2x mode)
                    m2 = m1_pool.tile([P, NCH // 2], bf16, tag="m2", name="m2",
                                      bufs=2)
                    nc.vector.tensor_tensor(
                        m2[:], xt[:, 0:NCH // 2], xt[:, NCH // 2:NCH], op=AL.max)
                    nc.vector.tensor_tensor(
                        mf[ct][:], mf[ct][:], m2[:], op=AL.max)
                    xb[ct][k] = xt

                # channel sum + gate logit rows via PE
                cg = ps_cg.tile([2, NCH], f32, tag="cg", name="cg")
                for s in range(NCH // 512):
                    ss = slice(s * 512, (s + 1) * 512)
                    for ct in range(NCT):
                        nc.tensor.matmul(
                            cg[:, ss], lhsT=og[ct][:], rhs=xb[ct][k][:, ss],
                            start=(ct == 0), stop=(ct == NCT - 1))
                # rows -> SBUF (bf16): row 0 = channel sum, row 1 = gate logit
                rows2 = rows_pool.tile([2, NCH], bf16, tag="rows2", name="rows2")
                nc.scalar.activation(rows2[:], cg[:], AF.Copy)
                # reshape rows into [h, w] maps (16 h-rows per chunk) via DRAM
                nc.sync.dma_start(cgrow_dram[b, k], rows2[:])
                hs = slice(k * (NCH // W), (k + 1) * (NCH // W))
                nc.sync.dma_start(
                    cs_hw[hs, :],
                    cgrow_dram[b, k, 0].rearrange("(h w) -> h w", w=W))
                nc.sync.dma_start(
                    glog_hw[hs, :],
                    cgrow_dram[b, k, 1].rearrange("(h w) -> h w", w=W))

                # channel max: pairwise then 32-block transpose-reduce
                m1 = m1_pool.tile([P, NCH], bf16, tag="m1", name="m1")
                nc.vector.tensor_tensor(m1[:], xb[0][k][:], xb[1][k][:], op=AL.max)
                nc.vector.tensor_reduce(
                    out=rmax[:, k * 64:(k + 1) * 64],
                    in_=m1[:].rearrange("p (j c) -> p j c", c=32),
                    axis=AX.X, op=AL.max, apply_transpose=True)

            # ---- finalize per-channel stats ----
            A_col, Dneg_col = [], []
            ssum = [stats.tile([P, 1], f32, tag=f"ssumf{ct}", name=f"ssumf{ct}") for ct in range(NCT)]
            smax = [stats.tile([P, 1], f32, tag=f"smaxf{ct}", name=f"smaxf{ct}") for ct in range(NCT)]
            for ct in range(NCT):
                nc.vector.tensor_reduce(
                    out=ssum[ct][:], in_=ssum_part[ct][:], axis=AX.X, op=AL.add)
                nc.vector.tensor_reduce(
                    out=smax[ct][:], in_=mf[ct][:], axis=AX.X, op=AL.max)
                # Dneg = -0.1 * alpha * avg = ssum * alpha * (-0.1/HW)
                dn = stats.tile([P, 1], f32, tag=f"dneg{ct}", name=f"dneg{ct}")
                nc.vector.tensor_scalar(
                    out=dn[:], in0=ssum[ct][:], scalar1=dnegs_col[:],
                    scalar2=None, op0=AL.mult)
                Dneg_col.append(dn)

            # ---- tiny MLP (shared_fc) on avg and mx ----
            obn = {}
            for name, vcols, lhsTs in (("A", ssum, w1Ts), ("M", smax, w1T)):
                hps = ps_mlp.tile([HID, 1], f32, tag="mlp_h", name="mlp_h")
                for ct in range(NCT):
                    nc.tensor.matmul(
                        hps[:], lhsT=lhsTs[ct][:], rhs=vcols[ct][:],
                        start=(ct == 0), stop=(ct == NCT - 1))
                h = stats.tile([HID, 1], f32, tag=f"h{name}", name=f"h{name}")
                nc.vector.tensor_copy(h[:], hps[:])
                d = stats.tile([HID, 1], f32, tag=f"d{name}", name=f"d{name}")
                nc.vector.tensor_tensor(d[:], h[:], p1mp2[:], op=AL.mult)
                sg = stats.tile([HID, 1], f32, tag=f"sg{name}", name=f"sg{name}")
                nc.scalar.activation(sg[:], d[:], AF.Sigmoid, scale=acbc[:])
                z = stats.tile([HID, 1], f32, tag=f"z{name}", name=f"z{name}")
                nc.vector.tensor_tensor(z[:], d[:], sg[:], op=AL.mult)
                h2 = stats.tile([HID, 1], f32, tag=f"h2{name}", name=f"h2{name}")
                nc.vector.scalar_tensor_tensor(
                    out=h2[:], in0=h[:], scalar=p2c[:], in1=z[:],
                    op0=AL.mult, op1=AL.add)
                for ct in range(NCT):
                    ops = ps_mlp.tile([P, 1], f32, tag="mlp_o", name="mlp_o")
                    nc.tensor.matmul(ops[:], lhsT=w2T[ct][:], rhs=h2[:],
                                     start=True, stop=True)
                    ob = stats.tile([P, 1], f32, tag=f"obn{name}{ct}", name=f"obn{name}{ct}")
                    nc.vector.scalar_tensor_tensor(
                        out=ob[:], in0=ops[:], scalar=bnscale[ct][:],
                        in1=bnbias[ct][:], op0=AL.mult, op1=AL.add)
                    obn[(name, ct)] = ob
            for ct in range(NCT):
                cap = stats.tile([P, 1], f32, tag=f"cap{ct}", name=f"cap{ct}")
                nc.vector.tensor_tensor(
                    cap[:], obn[("A", ct)][:], obn[("M", ct)][:], op=AL.add)
                sig = stats.tile([P, 1], f32, tag=f"sig{ct}", name=f"sig{ct}")
                nc.scalar.activation(sig[:], cap[:], AF.Sigmoid)
                ac = stats.tile([P, 1], f32, tag=f"acol{ct}", name=f"acol{ct}")
                nc.vector.scalar_tensor_tensor(
                    out=ac[:], in0=sig[:], scalar=alpha_col[:], in1=alpha01[:],
                    op0=AL.mult, op1=AL.add)
                A_col.append(ac)

            # ---- spatial attention maps ----
            # fold rmax [128, 512] (4 channel-groups) -> rr [32, 512]
            # (engine ops need matching start partitions: realign via DMA)
            ra = maps.tile([32, 3, 512], bf16, tag="ra", name="ra")
            for gi in range(3):
                nc.sync.dma_start(
                    ra[:, gi, :], rmax[32 * (gi + 1):32 * (gi + 2), :])
            r01 = maps.tile([32, 512], bf16, tag="r01", name="r01")
            nc.vector.tensor_tensor(r01[:], rmax[0:32, :], ra[:, 0, :], op=AL.max)
            r23 = maps.tile([32, 512], bf16, tag="r23", name="r23")
            nc.vector.tensor_tensor(r23[:], ra[:, 1, :], ra[:, 2, :], op=AL.max)
            rr = maps.tile([32, 512], bf16, tag="rr", name="rr")
            nc.vector.tensor_tensor(rr[:], r01[:], r23[:], op=AL.max)
            rrT = maps.tile([32, 512], bf16, tag="rrT", name="rrT")
            nc.vector.transpose(rrT[:], rr[:])
            # rrT[a, 32j+b] = chmax(pixel 1024j + 32a + b) -> smax_hw[h, w]
            # (via DRAM scratch; smax_hw[8j+a2, 32*a1+b] = rrT[4*a2+a1, 32j+b])
            nc.sync.dma_start(rrt_dram[b], rrT[:])
            smax_hw = maps.tile([P, W], bf16, tag="smax_hw", name="smax_hw")
            nc.sync.dma_start(
                smax_hw[:],
                rrt_dram[b].rearrange("(a2 a1) (j c) -> j a2 a1 c", a1=4, c=32))

            # gate map
            g_hw = maps.tile([P, W], bf16, tag="g_hw", name="g_hw")
            nc.scalar.activation(g_hw[:], glog_hw[:], AF.Sigmoid, bias=gateb_col[:])

            # 7x7 conv: zero-padded dy-shifted copies, then 98 fused taps
            shifts = {}
            for mi, mp_t in ((0, cs_hw), (1, smax_hw)):
                sh = maps.tile([P, 6 * W], bf16, tag=f"shift{mi}", name=f"shift{mi}")
                nc.vector.memset(sh[:], 0.0)
                slot = 0
                for dy in (-3, -2, -1, 1, 2, 3):
                    cslice = slice(slot * W, slot * W + W)
                    if dy < 0:
                        nc.sync.dma_start(sh[-dy:P, cslice], mp_t[0:P + dy, :])
                    else:
                        nc.sync.dma_start(sh[0:P - dy, cslice], mp_t[dy:P, :])
                    shifts[(mi, dy)] = sh[:, cslice]
                    slot += 1
                shifts[(mi, 0)] = mp_t[:]

            acc = maps.tile([P, W], bf16, tag="acc", name="acc")
            nc.vector.memset(acc[:], 0.0)
            for mi in range(2):
                for ky in range(7):
                    for kx in range(7):
                        dy, dx = ky - 3, kx - 3
                        widx = mi * 49 + ky * 7 + kx
                        src = shifts[(mi, dy)]
                        oc = slice(max(0, -dx), W - max(0, dx))
                        ic = slice(max(0, -dx) + dx, W - max(0, dx) + dx)
                        nc.vector.scalar_tensor_tensor(
                            out=acc[:, oc], in0=src[:, ic],
                            scalar=sa_cols[:, widx:widx + 1],
                            in1=acc[:, oc], op0=AL.mult, op1=AL.add)
            s_hw = maps.tile([P, W], bf16, tag="s_hw", name="s_hw")
            nc.scalar.activation(s_hw[:], acc[:], AF.Sigmoid, bias=sab_col[:])

            # q1 = 1 + beta * g * s
            q1a = maps.tile([P, W], bf16, tag="q1a", name="q1a")
            nc.vector.scalar_tensor_tensor(
                out=q1a[:], in0=s_hw[:], scalar=beta_col[:], in1=g_hw[:],
                op0=AL.mult, op1=AL.mult)
            q1_hw = maps.tile([P, W], bf16, tag="q1_hw", name="q1_hw")
            nc.vector.tensor_scalar_add(q1_hw[:], q1a[:], 1.0)

            # per-pixel rows to DRAM (for partition-broadcast reads)
            nc.sync.dma_start(
                rows_dram[b, 0, :].rearrange("(h w) -> h w", w=W), g_hw[:])
            nc.sync.dma_start(
                rows_dram[b, 1, :].rearrange("(h w) -> h w", w=W), q1_hw[:])

            # ---- output pass ----
            for k in range(NCHUNK):
                ks = slice(k * NCH, (k + 1) * NCH)
                gbt = bc_pool.tile([P, NCH], bf16, tag="gbt", name="gbt")
                nc.sync.dma_start(
                    gbt[:], rows_dram[b, 0, ks][None, :].to_broadcast([P, NCH]))
                q1t = bc_pool.tile([P, NCH], bf16, tag="q1t", name="q1t")
                nc.sync.dma_start(
                    q1t[:], rows_dram[b, 1, ks][None, :].to_broadcast([P, NCH]))
                for ct in range(NCT):
                    ts1 = work.tile([P, NCH], bf16, tag="ts1", name="ts1", bufs=1)
                    nc.vector.tensor_scalar(
                        out=ts1[:], in0=gbt[:], scalar1=A_col[ct][:],
                        scalar2=None, op0=AL.mult)
                    t = work.tile([P, NCH], bf16, tag="t", name="t")
                    nc.vector.tensor_tensor(t[:], ts1[:], q1t[:], op=AL.add)
                    u = work.tile([P, NCH], bf16, tag="u", name="u")
                    nc.vector.tensor_tensor(u[:], xb[ct][k][:], t[:], op=AL.mult)
                    v = work.tile([P, NCH], bf16, tag="v", name="v", bufs=1)
                    nc.scalar.activation(
                        v[:], gbt[:], AF.Copy, scale=Dneg_col[ct][:])
                    ob = work.tile([P, NCH], bf16, tag="ob", name="ob")
                    nc.vector.tensor_tensor(ob[:], u[:], v[:], op=AL.add)
                    of = of_pool.tile([P, NCH], f32, tag="of", name="of")
                    nc.scalar.activation(of[:], ob[:], AF.Copy)
                    nc.sync.dma_start(out_r[b, ct, :, ks], of[:])

    nc.compile()
    return nc


_NC_CACHE = None


def _get_nc():
    global _NC_CACHE
    if _NC_CACHE is None:
        _NC_CACHE = build_nc()
    return _NC_CACHE


def make_in_maps(inputs):
    f = np.float32
    w1 = np.asarray(inputs["w1"], f)                  # [HID, C]
    w2 = np.asarray(inputs["w2"], f)                  # [C, HID]
    p1 = np.asarray(inputs["p1"], f)
    p2 = np.asarray(inputs["p2"], f)
    aconb = np.asarray(inputs["acon_beta"], f)
    bn_g = np.asarray(inputs["bn_gamma"], f)
    bn_b = np.asarray(inputs["bn_beta"], f)
    bn_m = np.asarray(inputs["bn_mean"], f)
    bn_v = np.asarray(inputs["bn_var"], f)
    sa_w = np.asarray(inputs["sa_w"], f).reshape(98)
    sa_b = float(np.asarray(inputs["sa_b"], f).reshape(()))
    gate_w = np.asarray(inputs["gate_w"], f).reshape(C)
    gate_b = float(np.asarray(inputs["gate_b"], f).reshape(()))
    alpha = float(np.asarray(inputs["alpha"], f).reshape(()))
    beta = float(np.asarray(inputs["beta"], f).reshape(()))

    og = np.stack([np.ones(C, f), gate_w], axis=1)    # [C, 2]
    bnscale = bn_g / np.sqrt(bn_v + EPS)
    bnbias = bn_b - bn_m * bnscale
    sa_wp = sa_w.copy()
    sa_wp[:49] *= 1.0 / C                              # fold mean 1/C
    shared = {
        "og": np.ascontiguousarray(og),
        "w1t_avg": np.ascontiguousarray(w1.T / HW),
        "w1t": np.ascontiguousarray(w1.T),
        "w2t": np.ascontiguousarray(w2.T),
        "mlp_cols": np.ascontiguousarray(
            np.stack([p1 - p2, p2, aconb], axis=1)),
        "bn_cols": np.ascontiguousarray(
            np.stack([bnscale, bnbias], axis=1)),
        "sa_wp": sa_wp.reshape(1, 98),
        "sc_par": np.array(
            [[alpha, 0.1 * alpha, beta, gate_b, sa_b, -0.1 * alpha / HW]], f),
    }
    x = np.asarray(inputs["x"], f).reshape(B, C, HW)
    in_maps = []
    for i in range(NCORES):
        m = dict(shared)
        m["x"] = np.ascontiguousarray(x[i * BLOC:(i + 1) * BLOC])
        in_maps.append(m)
    return in_maps


def kernel(**inputs) -> np.ndarray:
    nc = _get_nc()
    in_maps = make_in_maps(inputs)
    res = run_bass_kernel_spmd(nc, in_maps, core_ids=list(range(NCORES)))
    out = np.concatenate([res.results[i]["out"] for i in range(NCORES)], axis=0)
    return out.reshape(B, C, H, W).astype(np.float32)


# revision 37
# speedup vs baseline: 1.2237x; 1.2237x over previous
"""Trainium2 Bass kernel for nn_AdaptiveAttention (dense_cnn).

Math (per image, C=256, H=W=128):
    avg = mean(x, spatial); mx = max(x, spatial)             [C]
    ca  = sigmoid(fc(avg) + fc(mx))                          [C]   (tiny MLP+BN)
    g   = sigmoid(gate_w . x + gate_b)                       [H,W]
    s   = sigmoid(conv7x7([mean_c(x), max_c(x)]) + sa_b)     [H,W]
    out = x*(1 + g*(alpha*ca + 0.1*alpha + beta*s)) - g*(0.1*alpha*avg)
        = x * t - g * D     with t = A_c*g + q1,  q1 = 1 + beta*g*s,
          A_c = alpha*ca_c + 0.1*alpha,  D_c = 0.1*alpha*avg_c

Distribution: pure data-parallel, 2 images per NeuronCore across 8 cores.
Core compute in bf16 (DVE 2x modes), stats accumulated in f32.
"""
import numpy as np
from contextlib import ExitStack

import concourse.bass as bass
import concourse.bacc as bacc
import concourse.mybir as mybir
import concourse.tile as tile
from concourse.bass_utils import run_bass_kernel_spmd

# ---- problem constants (hardcoded per spec) ----
B, C, H, W = 16, 256, 128, 128
NCORES = 8
BLOC = B // NCORES        # 2 images per core
HW = H * W                # 16384 pixels
P = 128                   # partitions
NCT = C // P              # 2 channel tiles
HID = 16
NCH = 2048                # pixels per chunk
NCHUNK = HW // NCH        # 8
EPS = 1e-5

f32 = mybir.dt.float32
bf16 = mybir.dt.bfloat16
AL = mybir.AluOpType
AF = mybir.ActivationFunctionType
AX = mybir.AxisListType


def build_nc():
    nc = bacc.Bacc()

    # ---- DRAM parameters ----
    x_ext = nc.declare_dram_parameter("x", [BLOC, C, HW], f32, isOutput=False)
    out_ext = nc.declare_dram_parameter("out", [BLOC, C, HW], f32, isOutput=True)
    # host-prepped parameters (see make_in_maps)
    og_ext = nc.declare_dram_parameter("og", [C, 2], f32, isOutput=False)
    w1ta_ext = nc.declare_dram_parameter("w1t_avg", [C, HID], f32, isOutput=False)
    w1t_ext = nc.declare_dram_parameter("w1t", [C, HID], f32, isOutput=False)
    w2t_ext = nc.declare_dram_parameter("w2t", [HID, C], f32, isOutput=False)
    mlpc_ext = nc.declare_dram_parameter("mlp_cols", [HID, 3], f32, isOutput=False)
    bnc_ext = nc.declare_dram_parameter("bn_cols", [C, 2], f32, isOutput=False)
    saw_ext = nc.declare_dram_parameter("sa_wp", [1, 98], f32, isOutput=False)
    # sc_par: [alpha, 0.1*alpha, beta, gate_b, sa_b, -0.1*alpha/HW]
    scp_ext = nc.declare_dram_parameter("sc_par", [1, 6], f32, isOutput=False)

    # DRAM scratch for per-pixel rows (g, q1) used for partition-broadcast
    rows_dram = nc.dram_tensor("rows_scratch", [BLOC, 2, HW], bf16)
    # DRAM scratch for the channel-max map rearrange
    rrt_dram = nc.dram_tensor("rrt_scratch", [BLOC, 32, 512], bf16)
    # DRAM scratch for channel-sum / gate-logit row reshapes
    cgrow_dram = nc.dram_tensor("cgrow_scratch", [BLOC, NCHUNK, 2, NCH], bf16)

    x_r = x_ext[:].rearrange("b (t p) n -> b t p n", p=P)
    out_r = out_ext[:].rearrange("b (t p) n -> b t p n", p=P)

    with tile.TileContext(nc) as tc, ExitStack() as ctx:
        const = ctx.enter_context(tc.tile_pool(name="const", bufs=1))
        stats = ctx.enter_context(tc.tile_pool(name="stats", bufs=2))
        maps = ctx.enter_context(tc.tile_pool(name="maps", bufs=2))
        xf_pool = ctx.enter_context(tc.tile_pool(name="xf", bufs=2))
        xb_pool = ctx.enter_context(tc.tile_pool(name="xb", bufs=2 * NCHUNK + 4))
        m1_pool = ctx.enter_context(tc.tile_pool(name="m1", bufs=1))
        rows_pool = ctx.enter_context(tc.tile_pool(name="rows", bufs=2))
        bc_pool = ctx.enter_context(tc.tile_pool(name="bc", bufs=2))
        work = ctx.enter_context(tc.tile_pool(name="work", bufs=2))
        of_pool = ctx.enter_context(tc.tile_pool(name="of", bufs=2))
        ps_cg = ctx.enter_context(tc.tile_pool(name="pscg", bufs=1, space="PSUM"))
        ps_mlp = ctx.enter_context(tc.tile_pool(name="psmlp", bufs=2, space="PSUM"))

        # ================= init: constants =================
        og = []
        for ct in range(NCT):
            of32 = const.tile([P, 2], f32, tag=f"ogf{ct}", name=f"ogf{ct}")
            nc.sync.dma_start(of32[:], og_ext[ct * P:(ct + 1) * P, :])
            o = const.tile([P, 2], bf16, tag=f"og{ct}", name=f"og{ct}")
            nc.vector.tensor_copy(o[:], of32[:])
            og.append(o)

        w1T, w1Ts, w2T = [], [], []
        for ct in range(NCT):
            cs = slice(ct * P, (ct + 1) * P)
            t = const.tile([P, HID], f32, tag=f"w1T{ct}", name=f"w1T{ct}")
            nc.sync.dma_start(t[:], w1t_ext[cs, :])
            ts_ = const.tile([P, HID], f32, tag=f"w1Ts{ct}", name=f"w1Ts{ct}")
            nc.sync.dma_start(ts_[:], w1ta_ext[cs, :])
            w2 = const.tile([HID, P], f32, tag=f"w2T{ct}", name=f"w2T{ct}")
            nc.sync.dma_start(w2[:], w2t_ext[:, cs])
            w1T.append(t)
            w1Ts.append(ts_)
            w2T.append(w2)

        mlpc = const.tile([HID, 3], f32, tag="mlpc", name="mlpc")
        nc.sync.dma_start(mlpc[:], mlpc_ext[:])
        p1mp2 = mlpc[:, 0:1]
        p2c = mlpc[:, 1:2]
        acbc = mlpc[:, 2:3]

        bnscale, bnbias = [], []
        for ct in range(NCT):
            cs = slice(ct * P, (ct + 1) * P)
            bc2 = const.tile([P, 2], f32, tag=f"bnc{ct}", name=f"bnc{ct}")
            nc.sync.dma_start(bc2[:], bnc_ext[cs, :])
            bnscale.append(bc2[:, 0:1])
            bnbias.append(bc2[:, 1:2])

        # broadcast columns [128, 1] from sc_par and the conv weights
        scp = const.tile([P, 6], f32, tag="scp", name="scp")
        nc.sync.dma_start(scp[:], scp_ext[:].to_broadcast([P, 6]))
        alpha_col = scp[:, 0:1]
        alpha01 = scp[:, 1:2]
        beta_col = scp[:, 2:3]
        gateb_col = scp[:, 3:4]
        sab_col = scp[:, 4:5]
        dnegs_col = scp[:, 5:6]
        sa_f32 = const.tile([P, 98], f32, tag="sa_f32", name="sa_f32")
        nc.sync.dma_start(sa_f32[:], saw_ext[:].to_broadcast([P, 98]))
        sa_cols = const.tile([P, 98], bf16, tag="sa_cols", name="sa_cols")
        nc.vector.tensor_copy(sa_cols[:], sa_f32[:])

        # ================= per-image pipeline =================
        for b in range(BLOC):
            ssum_part = [stats.tile([P, NCHUNK], f32, tag=f"ssum{ct}", name=f"ssum{ct}")
                         for ct in range(NCT)]
            smax_part = [stats.tile([P, NCHUNK], f32, tag=f"smax{ct}", name=f"smax{ct}")
                         for ct in range(NCT)]
            rmax = stats.tile([P, NCHUNK * 64], bf16, tag="rmax", name="rmax")
            cs_hw = maps.tile([P, W], bf16, tag="cs_hw", name="cs_hw")
            glog_hw = maps.tile([P, W], bf16, tag="glog_hw", name="glog_hw")

            xb = [[None] * NCHUNK for _ in range(NCT)]

            # ---- stats pass over chunks ----
            for k in range(NCHUNK):
                ks = slice(k * NCH, (k + 1) * NCH)
                for ct in range(NCT):
                    xf = xf_pool.tile([P, NCH], f32, tag="xf", name="xf")
                    nc.sync.dma_start(xf[:], x_r[b, ct, :, ks])
                    xt = xb_pool.tile([P, NCH], bf16, tag="xb", name="xb")
                    # fp32->bf16 convert; accumulate fp32 spatial sum for free
                    nc.scalar.activation(
                        out=xt[:], in_=xf[:], func=AF.Copy,
                        accum_out=ssum_part[ct][:, k:k + 1])
                    # spatial max of this chunk: pairwise TT fold (2x mode)
                    # then a half-size 1x reduce
                    m2 = m1_pool.tile([P, NCH // 2], bf16, tag="m2", name="m2", bufs=2)
                    nc.vector.tensor_tensor(
                        m2[:], xt[:, 0:NCH // 2], xt[:, NCH // 2:NCH], op=AL.max)
                    nc.vector.tensor_reduce(
                        out=smax_part[ct][:, k:k + 1], in_=m2[:],
                        axis=AX.X, op=AL.max)
                    xb[ct][k] = xt

                # channel sum + gate logit rows via PE
                cg = ps_cg.tile([2, NCH], f32, tag="cg", name="cg")
                for s in range(NCH // 512):
                    ss = slice(s * 512, (s + 1) * 512)
                    for ct in range(NCT):
                        nc.tensor.matmul(
                            cg[:, ss], lhsT=og[ct][:], rhs=xb[ct][k][:, ss],
                            start=(ct == 0), stop=(ct == NCT - 1))
                # rows -> SBUF (bf16): row 0 = channel sum, row 1 = gate logit
                rows2 = rows_pool.tile([2, NCH], bf16, tag="rows2", name="rows2")
                nc.scalar.activation(rows2[:], cg[:], AF.Copy)
                # reshape rows into [h, w] maps (16 h-rows per chunk) via DRAM
                nc.sync.dma_start(cgrow_dram[b, k], rows2[:])
                hs = slice(k * (NCH // W), (k + 1) * (NCH // W))
                nc.sync.dma_start(
                    cs_hw[hs, :],
                    cgrow_dram[b, k, 0].rearrange("(h w) -> h w", w=W))
                nc.sync.dma_start(
                    glog_hw[hs, :],
                    cgrow_dram[b, k, 1].rearrange("(h w) -> h w", w=W))

                # channel max: pairwise then 32-block transpose-reduce
                m1 = m1_pool.tile([P, NCH], bf16, tag="m1", name="m1")
                nc.vector.tensor_tensor(m1[:], xb[0][k][:], xb[1][k][:], op=AL.max)
                nc.vector.tensor_reduce(
                    out=rmax[:, k * 64:(k + 1) * 64],
                    in_=m1[:].rearrange("p (j c) -> p j c", c=32),
                    axis=AX.X, op=AL.max, apply_transpose=True)

            # ---- finalize per-channel stats ----
            A_col, Dneg_col = [], []
            ssum = [stats.tile([P, 1], f32, tag=f"ssumf{ct}", name=f"ssumf{ct}") for ct in range(NCT)]
            smax = [stats.tile([P, 1], f32, tag=f"smaxf{ct}", name=f"smaxf{ct}") for ct in range(NCT)]
            for ct in range(NCT):
                nc.vector.tensor_reduce(
                    out=ssum[ct][:], in_=ssum_part[ct][:], axis=AX.X, op=AL.add)
                nc.vector.tensor_reduce(
                    out=smax[ct][:], in_=smax_part[ct][:], axis=AX.X, op=AL.max)
                # Dneg = -0.1 * alpha * avg = ssum * alpha * (-0.1/HW)
                dn = stats.tile([P, 1], f32, tag=f"dneg{ct}", name=f"dneg{ct}")
                nc.vector.tensor_scalar(
                    out=dn[:], in0=ssum[ct][:], scalar1=dnegs_col[:],
                    scalar2=None, op0=AL.mult)
                Dneg_col.append(dn)

            # ---- tiny MLP (shared_fc) on avg and mx ----
            obn = {}
            for name, vcols, lhsTs in (("A", ssum, w1Ts), ("M", smax, w1T)):
                hps = ps_mlp.tile([HID, 1], f32, tag="mlp_h", name="mlp_h")
                for ct in range(NCT):
                    nc.tensor.matmul(
                        hps[:], lhsT=lhsTs[ct][:], rhs=vcols[ct][:],
                        start=(ct == 0), stop=(ct == NCT - 1))
                h = stats.tile([HID, 1], f32, tag=f"h{name}", name=f"h{name}")
                nc.vector.tensor_copy(h[:], hps[:])
                d = stats.tile([HID, 1], f32, tag=f"d{name}", name=f"d{name}")
                nc.vector.tensor_tensor(d[:], h[:], p1mp2[:], op=AL.mult)
                sg = stats.tile([HID, 1], f32, tag=f"sg{name}", name=f"sg{name}")
                nc.scalar.activation(sg[:], d[:], AF.Sigmoid, scale=acbc[:])
                z = stats.tile([HID, 1], f32, tag=f"z{name}", name=f"z{name}")
                nc.vector.tensor_tensor(z[:], d[:], sg[:], op=AL.mult)
                h2 = stats.tile([HID, 1], f32, tag=f"h2{name}", name=f"h2{name}")
                nc.vector.scalar_tensor_tensor(
                    out=h2[:], in0=h[:], scalar=p2c[:], in1=z[:],
                    op0=AL.mult, op1=AL.add)
                for ct in range(NCT):
                    ops = ps_mlp.tile([P, 1], f32, tag="mlp_o", name="mlp_o")
                    nc.tensor.matmul(ops[:], lhsT=w2T[ct][:], rhs=h2[:],
                                     start=True, stop=True)
                    ob = stats.tile([P, 1], f32, tag=f"obn{name}{ct}", name=f"obn{name}{ct}")
                    nc.vector.scalar_tensor_tensor(
                        out=ob[:], in0=ops[:], scalar=bnscale[ct][:],
                        in1=bnbias[ct][:], op0=AL.mult, op1=AL.add)
                    obn[(name, ct)] = ob
            for ct in range(NCT):
                cap = stats.tile([P, 1], f32, tag=f"cap{ct}", name=f"cap{ct}")
                nc.vector.tensor_tensor(
                    cap[:], obn[("A", ct)][:], obn[("M", ct)][:], op=AL.add)
                sig = stats.tile([P, 1], f32, tag=f"sig{ct}", name=f"sig{ct}")
                nc.scalar.activation(sig[:], cap[:], AF.Sigmoid)
                ac = stats.tile([P, 1], f32, tag=f"acol{ct}", name=f"acol{ct}")
                nc.vector.scalar_tensor_tensor(
                    out=ac[:], in0=sig[:], scalar=alpha_col[:], in1=alpha01[:],
                    op0=AL.mult, op1=AL.add)
                A_col.append(ac)

            # ---- spatial attention maps ----
            # fold rmax [128, 512] (4 channel-groups) -> rr [32, 512]
            # (engine ops need matching start partitions: realign via DMA)
            ra = maps.tile([32, 3, 512], bf16, tag="ra", name="ra")
            for gi in range(3):
                nc.sync.dma_start(
                    ra[:, gi, :], rmax[32 * (gi + 1):32 * (gi + 2), :])
            r01 = maps.tile([32, 512], bf16, tag="r01", name="r01")
            nc.vector.tensor_tensor(r01[:], rmax[0:32, :], ra[:, 0, :], op=AL.max)
            r23 = maps.tile([32, 512], bf16, tag="r23", name="r23")
            nc.vector.tensor_tensor(r23[:], ra[:, 1, :], ra[:, 2, :], op=AL.max)
            rr = maps.tile([32, 512], bf16, tag="rr", name="rr")
            nc.vector.tensor_tensor(rr[:], r01[:], r23[:], op=AL.max)
            rrT = maps.tile([32, 512], bf16, tag="rrT", name="rrT")
            nc.vector.transpose(rrT[:], rr[:])
            # rrT[a, 32j+b] = chmax(pixel 1024j + 32a + b) -> smax_hw[h, w]
            # (via DRAM scratch; smax_hw[8j+a2, 32*a1+b] = rrT[4*a2+a1, 32j+b])
            nc.sync.dma_start(rrt_dram[b], rrT[:])
            smax_hw = maps.tile([P, W], bf16, tag="smax_hw", name="smax_hw")
            nc.sync.dma_start(
                smax_hw[:],
                rrt_dram[b].rearrange("(a2 a1) (j c) -> j a2 a1 c", a1=4, c=32))

            # gate map
            g_hw = maps.tile([P, W], bf16, tag="g_hw", name="g_hw")
            nc.scalar.activation(g_hw[:], glog_hw[:], AF.Sigmoid, bias=gateb_col[:])

            # 7x7 conv: zero-padded dy-shifted copies, then 98 fused taps
            shifts = {}
            for mi, mp_t in ((0, cs_hw), (1, smax_hw)):
                sh = maps.tile([P, 6 * W], bf16, tag=f"shift{mi}", name=f"shift{mi}")
                nc.vector.memset(sh[:], 0.0)
                slot = 0
                for dy in (-3, -2, -1, 1, 2, 3):
                    cslice = slice(slot * W, slot * W + W)
                    if dy < 0:
                        nc.sync.dma_start(sh[-dy:P, cslice], mp_t[0:P + dy, :])
                    else:
                        nc.sync.dma_start(sh[0:P - dy, cslice], mp_t[dy:P, :])
                    shifts[(mi, dy)] = sh[:, cslice]
                    slot += 1
                shifts[(mi, 0)] = mp_t[:]

            acc = maps.tile([P, W], bf16, tag="acc", name="acc")
            nc.vector.memset(acc[:], 0.0)
            for mi in range(2):
                for ky in range(7):
                    for kx in range(7):
                        dy, dx = ky - 3, kx - 3
                        widx = mi * 49 + ky * 7 + kx
                        src = shifts[(mi, dy)]
                        oc = slice(max(0, -dx), W - max(0, dx))
                        ic = slice(max(0, -dx) + dx, W - max(0, dx) + dx)
                        nc.vector.scalar_tensor_tensor(
                            out=acc[:, oc], in0=src[:, ic],
                            scalar=sa_cols[:, widx:widx + 1],
                            in1=acc[:, oc], op0=AL.mult, op1=AL.add)
            s_hw = maps.tile([P, W], bf16, tag="s_hw", name="s_hw")
            nc.scalar.activation(s_hw[:], acc[:], AF.Sigmoid, bias=sab_col[:])

            # q1 = 1 + beta * g * s
            q1a = maps.tile([P, W], bf16, tag="q1a", name="q1a")
            nc.vector.scalar_tensor_tensor(
                out=q1a[:], in0=s_hw[:], scalar=beta_col[:], in1=g_hw[:],
                op0=AL.mult, op1=AL.mult)
            q1_hw = maps.tile([P, W], bf16, tag="q1_hw", name="q1_hw")
            nc.vector.tensor_scalar_add(q1_hw[:], q1a[:], 1.0)

            # per-pixel rows to DRAM (for partition-broadcast reads)
            nc.sync.dma_start(
                rows_dram[b, 0, :].rearrange("(h w) -> h w", w=W), g_hw[:])
            nc.sync.dma_start(
                rows_dram[b, 1, :].rearrange("(h w) -> h w", w=W), q1_hw[:])

            # ---- output pass ----
            for k in range(NCHUNK):
                ks = slice(k * NCH, (k + 1) * NCH)
                gbt = bc_pool.tile([P, NCH], bf16, tag="gbt", name="gbt")
                nc.sync.dma_start(
                    gbt[:], rows_dram[b, 0, ks][None, :].to_broadcast([P, NCH]))
                q1t = bc_pool.tile([P, NCH], bf16, tag="q1t", name="q1t")
                nc.sync.dma_start(
                    q1t[:], rows_dram[b, 1, ks][None, :].to_broadcast([P, NCH]))
                for ct in range(NCT):
                    ts1 = work.tile([P, NCH], bf16, tag="ts1", name="ts1", bufs=1)
                    nc.vector.tensor_scalar(
                        out=ts1[:], in0=gbt[:], scalar1=A_col[ct][:],
                        scalar2=None, op0=AL.mult)
                    t = work.tile([P, NCH], bf16, tag="t", name="t")
                    nc.vector.tensor_tensor(t[:], ts1[:], q1t[:], op=AL.add)
                    u = work.tile([P, NCH], bf16, tag="u", name="u")
                    nc.vector.tensor_tensor(u[:], xb[ct][k][:], t[:], op=AL.mult)
                    v = work.tile([P, NCH], bf16, tag="v", name="v", bufs=1)
                    nc.vector.tensor_scalar(
                        out=v[:], in0=gbt[:], scalar1=Dneg_col[ct][:],
                        scalar2=None, op0=AL.mult)
                    ob = work.tile([P, NCH], bf16, tag="ob", name="ob")
                    nc.vector.tensor_tensor(ob[:], u[:], v[:], op=AL.add)
                    of = of_pool.tile([P, NCH], f32, tag="of", name="of")
                    nc.scalar.activation(of[:], ob[:], AF.Copy)
                    nc.sync.dma_start(out_r[b, ct, :, ks], of[:])

    nc.compile()
    return nc


_NC_CACHE = None


def _get_nc():
    global _NC_CACHE
    if _NC_CACHE is None:
        _NC_CACHE = build_nc()
    return _NC_CACHE


def make_in_maps(inputs):
    f = np.float32
    w1 = np.asarray(inputs["w1"], f)                  # [HID, C]
    w2 = np.asarray(inputs["w2"], f)                  # [C, HID]
    p1 = np.asarray(inputs["p1"], f)
    p2 = np.asarray(inputs["p2"], f)
    aconb = np.asarray(inputs["acon_beta"], f)
    bn_g = np.asarray(inputs["bn_gamma"], f)
    bn_b = np.asarray(inputs["bn_beta"], f)
    bn_m = np.asarray(inputs["bn_mean"], f)
    bn_v = np.asarray(inputs["bn_var"], f)
    sa_w = np.asarray(inputs["sa_w"], f).reshape(98)
    sa_b = float(np.asarray(inputs["sa_b"], f).reshape(()))
    gate_w = np.asarray(inputs["gate_w"], f).reshape(C)
    gate_b = float(np.asarray(inputs["gate_b"], f).reshape(()))
    alpha = float(np.asarray(inputs["alpha"], f).reshape(()))
    beta = float(np.asarray(inputs["beta"], f).reshape(()))

    og = np.stack([np.ones(C, f), gate_w], axis=1)    # [C, 2]
    bnscale = bn_g / np.sqrt(bn_v + EPS)
    bnbias = bn_b - bn_m * bnscale
    sa_wp = sa_w.copy()
    sa_wp[:49] *= 1.0 / C                              # fold mean 1/C
    shared = {
        "og": np.ascontiguousarray(og),
        "w1t_avg": np.ascontiguousarray(w1.T / HW),
        "w1t": np.ascontiguousarray(w1.T),
        "w2t": np.ascontiguousarray(w2.T),
        "mlp_cols": np.ascontiguousarray(
            np.stack([p1 - p2, p2, aconb], axis=1)),
        "bn_cols": np.ascontiguousarray(
            np.stack([bnscale, bnbias], axis=1)),
        "sa_wp": sa_wp.reshape(1, 98),
        "sc_par": np.array(
            [[alpha, 0.1 * alpha, beta, gate_b, sa_b, -0.1 * alpha / HW]], f),
    }
    x = np.asarray(inputs["x"], f).reshape(B, C, HW)
    in_maps = []
    for i in range(NCORES):
        m = dict(shared)
        m["x"] = np.ascontiguousarray(x[i * BLOC:(i + 1) * BLOC])
        in_maps.append(m)
    return in_maps


def kernel(**inputs) -> np.ndarray:
    nc = _get_nc()
    in_maps = make_in_maps(inputs)
    res = run_bass_kernel_spmd(nc, in_maps, core_ids=list(range(NCORES)))
    out = np.concatenate([res.results[i]["out"] for i in range(NCORES)], axis=0)
    return out.reshape(B, C, H, W).astype(np.float32)
